# revision 34
# baseline (speedup 1.0000x reference)
"""Int8DynActInt8WeightLinear on 8 trn2 NeuronCores.

Math (exact-integer reformulation of the reference):
  per token t: sc_t = max((mx-mn)/255, eps)  (mn<=0<=mx for randn inputs)
  A[t,i] = round(x[t,i]/sc_t)   (integer in [-255,255], bf16-exact)
  y[t,o] = sc_t * (A @ W^T - Asum_t * z_o) * s_o
For og>=2 the weights are shipped as uint8 (W+128) and cast to bf16 by the
SWDGE dma; the +128 is absorbed exactly by the zeros correction (zb+128).

Per core (data-parallel over tokens, 1024 tokens/core):
  - x loaded in 4 quarter-chunks per 128-token tile (SWDGE queue); DVE does
    min/max per quarter; the tiny combine chain (trees, d, sc, rinv=255/d)
    runs on GpSimd so it never queues behind DVE's big reduces.
  - round(x*rinv) via the two-pass +-2^23*1.5 trick: pass1 on GpSimd
    (tensor_scalar with rinv AP scalar + TWO23 immediate), pass2 on ScalarE
    per quarter (emits bf16 A + Asum via accum_out).
  - transpose to at[p,k,q]=A[q,128k+p]: tiles 0-1 via PE is_transpose
    matmuls (8 per quarter into a PSUM bank, DVE copies PSUM->SBUF) so the
    first matmuls start ~15us; tiles 2-7 via one-shot xbar DMA transpose
    (latency hidden behind the og0/og1 phase-1 consumption).
  - matmul: 8 output groups of 512 wide; og0/og1 weights are bf16 on the
    HWDGE (scalar) queue so they stream concurrently with x at startup;
    og2-7 are uint8 cast-DMAs (halves their HBM traffic). Phase 1
    interleaves og0/og1 per t-tile; og0 retires early so og2's DMA hides.
  - epilogue on DVE: c1 = ps - Asum*z, y = (c1*sc)*s emitted as bf16
    (halves the output DMA; host upcasts to f32).
"""

import sys

sys.path.insert(0, "/opt/trn_rl_repo")

import numpy as np
import ml_dtypes

import concourse.bass as bass
import concourse.mybir as mybir
import concourse.tile as tile
from concourse import bacc
from concourse.bass_utils import run_bass_kernel_spmd

F32 = mybir.dt.float32
BF16 = mybir.dt.bfloat16
U8 = mybir.dt.uint8
X = mybir.AxisListType.X
OP = mybir.AluOpType
ACT = mybir.ActivationFunctionType

P = 128
IN = 4096
OUT = 4096
NK = 32          # contraction k-tiles of 128
T = 1024         # tokens per core
NT = T // P      # 8 t-tiles
OGW = 512        # output-group width (one PSUM bank)
NOG = OUT // OGW # 8
NQ = 4           # x quarter-chunks per tile
QW = IN // NQ    # 1024
KQ = NK // NQ    # 8 k-tiles per quarter
TWO23 = 12582912.0  # 1.5*2^23: v+TWO23 rounds v to int for |v|<2^22
EPS = float(np.finfo(np.float32).eps)
NCORES = 8
N_PETP = 4       # tiles transposed on the PE (rest via xbar DMA)


def _build_nc():
    nc = bacc.Bacc("TRN2", target_bir_lowering=False, debug=False)
    x_d = nc.dram_tensor("x", [T, IN], F32, kind="ExternalInput")
    # wt[og*P + p, k*OGW + j] = W[og*OGW + j, k*P + p]  (og 0/1, bf16)
    wt01_d = nc.dram_tensor("wt01", [2 * P, NK * OGW], BF16,
                            kind="ExternalInput")
    # og 2..7 as uint8 = W + 128
    wtu8_d = nc.dram_tensor("wtu8", [(NOG - 2) * P, NK * OGW], U8,
                            kind="ExternalInput")
    sb_d = nc.dram_tensor("sb", [P, OUT], BF16, kind="ExternalInput")
    # zb columns of og>=2 carry z+128 (absorbs the uint8 weight offset);
    # all zb values are small integers, exact in bf16
    zb_d = nc.dram_tensor("zb", [P, OUT], BF16, kind="ExternalInput")
    id_d = nc.dram_tensor("ident", [P, P], BF16, kind="ExternalInput")
    y_d = nc.dram_tensor("y", [T, OUT], BF16, kind="ExternalOutput")

    x_t = x_d[:].rearrange("(nt p) i -> nt p i", p=P)      # [NT, 128, IN]
    y_t = y_d[:].rearrange("(nt p) o -> nt p o", p=P)      # [NT, 128, OUT]

    with tile.TileContext(nc) as tc:
        with (
            tc.tile_pool(name="xq", bufs=8) as xqp,
            tc.tile_pool(name="apool", bufs=2) as apool,
            tc.tile_pool(name="atpool", bufs=NT) as atpool,
            tc.tile_pool(name="wpool", bufs=2) as wpool,
            tc.tile_pool(name="scp", bufs=NT) as scp,
            tc.tile_pool(name="nasp", bufs=NT) as nasp,
            tc.tile_pool(name="stats", bufs=40) as stats,
            tc.tile_pool(name="consts", bufs=3) as consts,
            tc.tile_pool(name="sbp", bufs=2) as sbp,
            tc.tile_pool(name="zbp", bufs=2) as zbp,
            tc.tile_pool(name="cpool", bufs=2) as cpool,
            tc.tile_pool(name="ypool", bufs=3) as ypool,
            tc.tile_pool(name="mmps", bufs=6, space="PSUM") as mmps,
            tc.tile_pool(name="tpps", bufs=2, space="PSUM") as tpps,
        ):
            wg_tiles = {}
            sbs_tiles = {}
            zbs_tiles = {}
            at_tiles = [None] * NT
            sc_tiles = [None] * NT
            nas_tiles = [None] * NT
            a_tiles = [None] * NT

            ident = consts.tile([P, P], BF16, tag="ident")
            nc.sync.dma_start(ident[:], id_d[:])

            def quant_tile(tt, do_transpose=True):
                """x DMA, minmax, rinv, 2-pass round, asum for tile tt."""
                xs = []
                for c in range(NQ):
                    xc = xqp.tile([P, QW], F32, tag="xq")
                    nc.sync.dma_start(
                        xc[:], x_t[tt, :, c * QW:(c + 1) * QW])
                    xs.append(xc)

                if tt < 2:
                    # og0 (tt==0) / og1 (tt==1) weights in four 1MB column
                    # (k-range) chunks on the HWDGE scalar queue.  Chunk c
                    # is gated on x-quarter c of this tile via a 1-elem
                    # copy so the weight stream cannot crowd the x stream
                    # out of HBM bandwidth (HWDGE otherwise wins the
                    # arbitration ~5:1 and the whole quant pipeline slides
                    # ~30us).  og0's first chunk is ungated.
                    og = tt
                    wg = wpool.tile([P, NK * OGW], BF16, tag="wg")
                    qc = (NK // NQ) * OGW
                    for c in range(NQ):
                        cols = slice(c * qc, (c + 1) * qc)
                        nc.scalar.copy(wg[0:1, c * qc:c * qc + 1],
                                       xs[c][0:1, 0:1])
                        nc.scalar.dma_start(
                            wg[:, cols], wt01_d[og * P:(og + 1) * P, cols])
                    wg_tiles[og] = wg
                    # aux gated on this tile's last x quarter: nothing may
                    # crowd the x stream out of the DMA engines at startup
                    osl = slice(og * OGW, (og + 1) * OGW)
                    sbs = sbp.tile([P, OGW], BF16, tag="sbs")
                    nc.gpsimd.tensor_copy(sbs[0:1, 0:1], xs[3][0:1, 0:1])
                    nc.gpsimd.dma_start(sbs[:], sb_d[:, osl])
                    zbs = zbp.tile([P, OGW], BF16, tag="zbs")
                    nc.gpsimd.tensor_copy(zbs[0:1, 0:1], xs[3][0:1, 0:1])
                    nc.gpsimd.dma_start(zbs[:], zb_d[:, osl])
                    sbs_tiles[og] = sbs
                    zbs_tiles[og] = zbs

                # quarter min/max land in adjacent columns of one [P,8]
                # tile; the combine chain is then just 2 tiny reduces.
                st8 = stats.tile([P, 8], F32, tag="st8")
                for c in range(NQ):
                    nc.vector.tensor_reduce(st8[:, c:c + 1], xs[c][:],
                                            axis=X, op=OP.min)
                    nc.vector.tensor_reduce(st8[:, 4 + c:5 + c], xs[c][:],
                                            axis=X, op=OP.max)

                with tc.high_priority(offset=60):
                    mn = stats.tile([P, 1], F32, tag="mn")
                    mx = stats.tile([P, 1], F32, tag="mx")
                    nc.vector.tensor_reduce(mn[:], st8[:, 0:4], axis=X,
                                            op=OP.min)
                    nc.vector.tensor_reduce(mx[:], st8[:, 4:8], axis=X,
                                            op=OP.max)
                    d = stats.tile([P, 1], F32, tag="d")
                    nc.vector.tensor_tensor(d[:], mx[:], mn[:],
                                            op=OP.subtract)
                    sc = scp.tile([P, 1], F32, tag="sc")
                    nc.vector.tensor_scalar(sc[:], d[:], 1.0 / 255.0, EPS,
                                            op0=OP.mult, op1=OP.max)
                    rinv = stats.tile([P, 1], F32, tag="rinv")
                    nc.vector.reciprocal(rinv[:], sc[:])

                # two-pass round: pass1 (x*rinv + TWO23) on DVE for the
                # first two tiles (startup latency; ScalarE is the serial
                # bottleneck there) and on ScalarE afterwards (the DVE is
                # the steady-state bottleneck); pass2 (-TWO23, bf16 out,
                # Asum accum) on ScalarE.
                a_tile = apool.tile([P, IN], BF16, tag="a")
                asum4 = stats.tile([P, NQ], F32, tag="asum4")
                for c in range(NQ):
                    if tt < 2:
                        nc.vector.tensor_scalar(
                            xs[c][:], xs[c][:], rinv[:], TWO23,
                            op0=OP.mult, op1=OP.add)
                    else:
                        nc.scalar.activation(
                            xs[c][:], xs[c][:], ACT.Copy,
                            bias=TWO23, scale=rinv[:])
                    nc.scalar.activation(
                        a_tile[:, c * QW:(c + 1) * QW], xs[c][:],
                        ACT.Copy, bias=-TWO23, scale=1.0,
                        accum_out=asum4[:, c:c + 1])

                nas = nasp.tile([P, 1], F32, tag="nas")
                nc.vector.tensor_reduce(nas[:], asum4[:], axis=X, op=OP.add,
                                        negate=True)

                a_tiles[tt] = a_tile
                sc_tiles[tt] = sc
                nas_tiles[tt] = nas
                at = atpool.tile([P, NK, P], BF16, tag="at")
                at_tiles[tt] = at
                if do_transpose:
                    emit_transpose(tt)

            def emit_transpose(tt):
                at = at_tiles[tt]
                a_tile = a_tiles[tt]
                if tt < N_PETP:
                    # PE transpose, one PSUM bank per quarter (8 k-tiles),
                    # DVE copies the bank to SBUF.
                    for c in range(NQ):
                        tp = tpps.tile([P, KQ, P], BF16, tag="tp")
                        for k8 in range(KQ):
                            k = c * KQ + k8
                            nc.tensor.transpose(
                                tp[:, k8, :],
                                a_tile[:, k * P:(k + 1) * P], ident[:])
                        with tc.high_priority(offset=40):
                            nc.vector.tensor_copy(
                                at[:, c * KQ:(c + 1) * KQ, :], tp[:])
                else:
                    nc.sync.dma_start_transpose(at[:], a_tile[:])

            def matmul_group(og, tt):
                if tt == 0 and og >= 2:
                    osl = slice(og * OGW, (og + 1) * OGW)
                    sbs = sbp.tile([P, OGW], BF16, tag="sbs")
                    nc.scalar.dma_start(sbs[:], sb_d[:, osl])
                    zbs = zbp.tile([P, OGW], BF16, tag="zbs")
                    nc.scalar.dma_start(zbs[:], zb_d[:, osl])
                    sbs_tiles[og] = sbs
                    zbs_tiles[og] = zbs
                    # uint8 weights, cast to bf16 by the SWDGE dma; two
                    # halves so the og's first k-groups can start while the
                    # second half is still streaming
                    wg = wpool.tile([P, NK * OGW], BF16, tag="wg")
                    half = (NK // 2) * OGW
                    rows = slice((og - 2) * P, (og - 1) * P)
                    nc.gpsimd.dma_start(wg[:, :half], wtu8_d[rows, :half])
                    nc.gpsimd.dma_start(wg[:, half:], wtu8_d[rows, half:])
                    wg_tiles[og] = wg
                osl = slice(og * OGW, (og + 1) * OGW)
                wg = wg_tiles[og]
                ps = mmps.tile([P, OGW], F32, tag="ps")
                for k in range(NK):
                    nc.tensor.matmul(ps[:], at_tiles[tt][:, k, :],
                                     wg[:, k * OGW:(k + 1) * OGW],
                                     start=(k == 0), stop=(k == NK - 1))
                # c1 = ps - Asum*z ; y = (c1*sc)*s  (bf16 epilogue tiles:
                # c1 rounding ~2e-3 rel, well inside the 2e-2 gate, and
                # 16-bit operands speed up the DVE)
                c1 = cpool.tile([P, OGW], BF16, tag="c1")
                nc.vector.scalar_tensor_tensor(
                    c1[:], zbs_tiles[og][:], nas_tiles[tt][:], ps[:],
                    op0=OP.mult, op1=OP.add)
                y2 = ypool.tile([P, OGW], BF16, tag="y2")
                nc.vector.scalar_tensor_tensor(
                    y2[:], c1[:], sc_tiles[tt][:], sbs_tiles[og][:],
                    op0=OP.mult, op1=OP.mult)
                nc.scalar.dma_start(y_t[tt, :, osl], y2[:])

            # ---------------- emission order ----------------
            # The PE queue must interleave the early tiles' PE transposes
            # with the first matmul groups (in-order queue: a transpose
            # emitted too early would stall matmuls behind its readiness).
            # Phase 1 runs og0/og1 PAIRED per t-tile, which pushes every
            # at-tile's deadline one 7us slot later than the lead-lag
            # order and lets the quant pipeline absorb transpose latency.
            quant_tile(0, do_transpose=True)
            quant_tile(1, do_transpose=False)
            matmul_group(0, 0)
            emit_transpose(1)
            quant_tile(2, do_transpose=False)
            matmul_group(1, 0)
            emit_transpose(2)
            quant_tile(3, do_transpose=False)
            matmul_group(0, 1)
            emit_transpose(3)
            for tt in range(4, NT):
                quant_tile(tt, do_transpose=True)

            # og0 finishes two slots early ((0,7) hoisted before (1,6)) so
            # og2's weight dma (ring-gated on og0's tile) hides behind
            # og1's tail groups.
            seq = [(1, 1)]
            for tt in range(2, NT - 1):
                seq += [(0, tt), (1, tt)]
            seq.remove((1, NT - 2))
            seq += [(0, NT - 1), (1, NT - 2), (1, NT - 1)]
            for og in range(2, NOG):
                seq += [(og, tt) for tt in range(NT)]
            for og, tt in seq:
                matmul_group(og, tt)

    nc.compile()
    return nc


_NC = None


def _get_nc():
    global _NC
    if _NC is None:
        _NC = _build_nc()
    return _NC


def _prep_inputs(x, weight, scales, zeros):
    x2 = np.ascontiguousarray(x.reshape(NCORES * T, IN).astype(np.float32))
    w4 = weight.astype(np.float32).reshape(NOG, OGW, NK, P)
    wt = np.ascontiguousarray(
        w4.transpose(0, 3, 2, 1).reshape(NOG * P, NK * OGW))
    wt01 = wt[:2 * P].astype(ml_dtypes.bfloat16)
    wtu8 = (wt[2 * P:] + 128.0).astype(np.uint8)
    sb = np.ascontiguousarray(np.broadcast_to(
        scales.astype(np.float32), (P, OUT))).astype(ml_dtypes.bfloat16)
    zadj = zeros.astype(np.float32).copy()
    zadj[2 * OGW:] += 128.0
    zb = np.ascontiguousarray(
        np.broadcast_to(zadj, (P, OUT))).astype(ml_dtypes.bfloat16)
    ident = np.eye(P, dtype=ml_dtypes.bfloat16)
    in_maps = []
    for c in range(NCORES):
        in_maps.append({
            "x": np.ascontiguousarray(x2[c * T:(c + 1) * T]),
            "wt01": wt01,
            "wtu8": wtu8,
            "sb": sb,
            "zb": zb,
            "ident": ident,
        })
    return in_maps


def _run(x, weight, scales, zeros, trace=False):
    nc = _get_nc()
    in_maps = _prep_inputs(x, weight, scales, zeros)
    bkr = run_bass_kernel_spmd(nc, in_maps, core_ids=list(range(NCORES)),
                               trace=trace)
    y = np.concatenate([np.asarray(r["y"]) for r in bkr.results], axis=0)
    y = y.astype(np.float32).reshape(4, 2048, OUT)
    return y, bkr


def kernel(x, weight, scales, zeros):
    y, _ = _run(x, weight, scales, zeros, trace=False)
    return y


# revision 38
# speedup vs baseline: 1.0026x; 1.0026x over previous
"""Int8DynActInt8WeightLinear on 8 trn2 NeuronCores.

Math (exact-integer reformulation of the reference):
  per token t: sc_t = max((mx-mn)/255, eps)  (mn<=0<=mx for randn inputs)
  A[t,i] = round(x[t,i]/sc_t)   (integer in [-255,255], bf16-exact)
  y[t,o] = sc_t * (A @ W^T - Asum_t * z_o) * s_o
For og>=2 the weights are shipped as uint8 (W+128) and cast to bf16 by the
SWDGE dma; the +128 is absorbed exactly by the zeros correction (zb+128).

Per core (data-parallel over tokens, 1024 tokens/core):
  - x loaded in 4 quarter-chunks per 128-token tile (SWDGE queue); DVE does
    min/max per quarter; the tiny combine chain (trees, d, sc, rinv=255/d)
    runs on GpSimd so it never queues behind DVE's big reduces.
  - round(x*rinv) via the two-pass +-2^23*1.5 trick: pass1 on GpSimd
    (tensor_scalar with rinv AP scalar + TWO23 immediate), pass2 on ScalarE
    per quarter (emits bf16 A + Asum via accum_out).
  - transpose to at[p,k,q]=A[q,128k+p]: tiles 0-1 via PE is_transpose
    matmuls (8 per quarter into a PSUM bank, DVE copies PSUM->SBUF) so the
    first matmuls start ~15us; tiles 2-7 via one-shot xbar DMA transpose
    (latency hidden behind the og0/og1 phase-1 consumption).
  - matmul: 8 output groups of 512 wide; og0/og1 weights are bf16 on the
    HWDGE (scalar) queue so they stream concurrently with x at startup;
    og2-7 are uint8 cast-DMAs (halves their HBM traffic). Phase 1
    interleaves og0/og1 per t-tile; og0 retires early so og2's DMA hides.
  - epilogue on DVE: c1 = ps - Asum*z, y = (c1*sc)*s emitted as bf16
    (halves the output DMA; host upcasts to f32).
"""

import sys

sys.path.insert(0, "/opt/trn_rl_repo")

import numpy as np
import ml_dtypes

import concourse.bass as bass
import concourse.mybir as mybir
import concourse.tile as tile
from concourse import bacc
from concourse.bass_utils import run_bass_kernel_spmd

F32 = mybir.dt.float32
BF16 = mybir.dt.bfloat16
U8 = mybir.dt.uint8
X = mybir.AxisListType.X
OP = mybir.AluOpType
ACT = mybir.ActivationFunctionType

P = 128
IN = 4096
OUT = 4096
NK = 32          # contraction k-tiles of 128
T = 1024         # tokens per core
NT = T // P      # 8 t-tiles
OGW = 512        # output-group width (one PSUM bank)
NOG = OUT // OGW # 8
NQ = 4           # x quarter-chunks per tile
QW = IN // NQ    # 1024
KQ = NK // NQ    # 8 k-tiles per quarter
TWO23 = 12582912.0  # 1.5*2^23: v+TWO23 rounds v to int for |v|<2^22
EPS = float(np.finfo(np.float32).eps)
NCORES = 8
N_PETP = 5       # tiles transposed on the PE (rest via xbar DMA)


def _build_nc():
    nc = bacc.Bacc("TRN2", target_bir_lowering=False, debug=False)
    x_d = nc.dram_tensor("x", [T, IN], F32, kind="ExternalInput")
    # wt[og*P + p, k*OGW + j] = W[og*OGW + j, k*P + p]  (og 0/1, bf16)
    wt01_d = nc.dram_tensor("wt01", [2 * P, NK * OGW], BF16,
                            kind="ExternalInput")
    # og 2..7 as uint8 = W + 128
    wtu8_d = nc.dram_tensor("wtu8", [(NOG - 2) * P, NK * OGW], U8,
                            kind="ExternalInput")
    sb_d = nc.dram_tensor("sb", [P, OUT], BF16, kind="ExternalInput")
    # zb columns of og>=2 carry z+128 (absorbs the uint8 weight offset);
    # all zb values are small integers, exact in bf16
    zb_d = nc.dram_tensor("zb", [P, OUT], BF16, kind="ExternalInput")
    id_d = nc.dram_tensor("ident", [P, P], BF16, kind="ExternalInput")
    y_d = nc.dram_tensor("y", [T, OUT], BF16, kind="ExternalOutput")

    x_t = x_d[:].rearrange("(nt p) i -> nt p i", p=P)      # [NT, 128, IN]
    y_t = y_d[:].rearrange("(nt p) o -> nt p o", p=P)      # [NT, 128, OUT]

    with tile.TileContext(nc) as tc:
        with (
            tc.tile_pool(name="xq", bufs=8) as xqp,
            tc.tile_pool(name="apool", bufs=2) as apool,
            tc.tile_pool(name="atpool", bufs=NT) as atpool,
            tc.tile_pool(name="wpool", bufs=2) as wpool,
            tc.tile_pool(name="scp", bufs=NT) as scp,
            tc.tile_pool(name="nasp", bufs=NT) as nasp,
            tc.tile_pool(name="stats", bufs=40) as stats,
            tc.tile_pool(name="consts", bufs=3) as consts,
            tc.tile_pool(name="sbp", bufs=2) as sbp,
            tc.tile_pool(name="zbp", bufs=2) as zbp,
            tc.tile_pool(name="cpool", bufs=2) as cpool,
            tc.tile_pool(name="ypool", bufs=3) as ypool,
            tc.tile_pool(name="mmps", bufs=6, space="PSUM") as mmps,
            tc.tile_pool(name="tpps", bufs=2, space="PSUM") as tpps,
        ):
            wg_tiles = {}
            sbs_tiles = {}
            zbs_tiles = {}
            at_tiles = [None] * NT
            sc_tiles = [None] * NT
            nas_tiles = [None] * NT
            a_tiles = [None] * NT

            ident = consts.tile([P, P], BF16, tag="ident")
            nc.sync.dma_start(ident[:], id_d[:])

            def quant_tile(tt, do_transpose=True):
                """x DMA, minmax, rinv, 2-pass round, asum for tile tt."""
                xs = []
                for c in range(NQ):
                    xc = xqp.tile([P, QW], F32, tag="xq")
                    nc.sync.dma_start(
                        xc[:], x_t[tt, :, c * QW:(c + 1) * QW])
                    xs.append(xc)

                if tt < 2:
                    # og0 (tt==0) / og1 (tt==1) weights in four 1MB column
                    # (k-range) chunks on the HWDGE scalar queue.  Chunk c
                    # is gated on x-quarter c of this tile via a 1-elem
                    # copy so the weight stream cannot crowd the x stream
                    # out of HBM bandwidth (HWDGE otherwise wins the
                    # arbitration ~5:1 and the whole quant pipeline slides
                    # ~30us).  og0's first chunk is ungated.
                    og = tt
                    wg = wpool.tile([P, NK * OGW], BF16, tag="wg")
                    qc = (NK // NQ) * OGW
                    for c in range(NQ):
                        cols = slice(c * qc, (c + 1) * qc)
                        # og0's chunks gate 2 quarters later than og1's so
                        # x0 gets the DMA engines to itself first
                        gate = xs[min(c + 2, NQ - 1)] if tt == 0 else xs[c]
                        nc.scalar.copy(wg[0:1, c * qc:c * qc + 1],
                                       gate[0:1, 0:1])
                        nc.scalar.dma_start(
                            wg[:, cols], wt01_d[og * P:(og + 1) * P, cols])
                    wg_tiles[og] = wg
                    # aux gated on this tile's last x quarter: nothing may
                    # crowd the x stream out of the DMA engines at startup
                    osl = slice(og * OGW, (og + 1) * OGW)
                    sbs = sbp.tile([P, OGW], BF16, tag="sbs")
                    nc.gpsimd.tensor_copy(sbs[0:1, 0:1], xs[3][0:1, 0:1])
                    nc.gpsimd.dma_start(sbs[:], sb_d[:, osl])
                    zbs = zbp.tile([P, OGW], BF16, tag="zbs")
                    nc.gpsimd.tensor_copy(zbs[0:1, 0:1], xs[3][0:1, 0:1])
                    nc.gpsimd.dma_start(zbs[:], zb_d[:, osl])
                    sbs_tiles[og] = sbs
                    zbs_tiles[og] = zbs

                # quarter min/max land in adjacent columns of one [P,8]
                # tile; the combine chain is then just 2 tiny reduces.
                st8 = stats.tile([P, 8], F32, tag="st8")
                for c in range(NQ):
                    nc.vector.tensor_reduce(st8[:, c:c + 1], xs[c][:],
                                            axis=X, op=OP.min)
                    nc.vector.tensor_reduce(st8[:, 4 + c:5 + c], xs[c][:],
                                            axis=X, op=OP.max)

                with tc.high_priority():
                    mn = stats.tile([P, 1], F32, tag="mn")
                    mx = stats.tile([P, 1], F32, tag="mx")
                    nc.vector.tensor_reduce(mn[:], st8[:, 0:4], axis=X,
                                            op=OP.min)
                    nc.vector.tensor_reduce(mx[:], st8[:, 4:8], axis=X,
                                            op=OP.max)
                    d = stats.tile([P, 1], F32, tag="d")
                    nc.vector.tensor_tensor(d[:], mx[:], mn[:],
                                            op=OP.subtract)
                    sc = scp.tile([P, 1], F32, tag="sc")
                    nc.vector.tensor_scalar(sc[:], d[:], 1.0 / 255.0, EPS,
                                            op0=OP.mult, op1=OP.max)
                    rinv = stats.tile([P, 1], F32, tag="rinv")
                    nc.vector.reciprocal(rinv[:], sc[:])

                # two-pass round: pass1 (x*rinv + TWO23) on DVE for the
                # first two tiles (startup latency; ScalarE is the serial
                # bottleneck there) and on ScalarE afterwards (the DVE is
                # the steady-state bottleneck); pass2 (-TWO23, bf16 out,
                # Asum accum) on ScalarE.
                a_tile = apool.tile([P, IN], BF16, tag="a")
                asum4 = stats.tile([P, NQ], F32, tag="asum4")
                for c in range(NQ):
                    if tt < 2:
                        nc.vector.tensor_scalar(
                            xs[c][:], xs[c][:], rinv[:], TWO23,
                            op0=OP.mult, op1=OP.add)
                    else:
                        nc.scalar.activation(
                            xs[c][:], xs[c][:], ACT.Copy,
                            bias=TWO23, scale=rinv[:])
                    nc.scalar.activation(
                        a_tile[:, c * QW:(c + 1) * QW], xs[c][:],
                        ACT.Copy, bias=-TWO23, scale=1.0,
                        accum_out=asum4[:, c:c + 1])

                nas = nasp.tile([P, 1], F32, tag="nas")
                nc.vector.tensor_reduce(nas[:], asum4[:], axis=X, op=OP.add,
                                        negate=True)

                a_tiles[tt] = a_tile
                sc_tiles[tt] = sc
                nas_tiles[tt] = nas
                at = atpool.tile([P, NK, P], BF16, tag="at")
                at_tiles[tt] = at
                if do_transpose:
                    emit_transpose(tt)

            def emit_transpose(tt):
                at = at_tiles[tt]
                a_tile = a_tiles[tt]
                if tt < N_PETP:
                    # PE transpose, one PSUM bank per quarter (8 k-tiles),
                    # DVE copies the bank to SBUF.
                    for c in range(NQ):
                        tp = tpps.tile([P, KQ, P], BF16, tag="tp")
                        for k8 in range(KQ):
                            k = c * KQ + k8
                            nc.tensor.transpose(
                                tp[:, k8, :],
                                a_tile[:, k * P:(k + 1) * P], ident[:])
                        with tc.high_priority(offset=40):
                            nc.vector.tensor_copy(
                                at[:, c * KQ:(c + 1) * KQ, :], tp[:])
                else:
                    nc.sync.dma_start_transpose(at[:], a_tile[:])

            def matmul_group(og, tt):
                if tt == 0 and og >= 2:
                    osl = slice(og * OGW, (og + 1) * OGW)
                    sbs = sbp.tile([P, OGW], BF16, tag="sbs")
                    nc.scalar.dma_start(sbs[:], sb_d[:, osl])
                    zbs = zbp.tile([P, OGW], BF16, tag="zbs")
                    nc.scalar.dma_start(zbs[:], zb_d[:, osl])
                    sbs_tiles[og] = sbs
                    zbs_tiles[og] = zbs
                    # uint8 weights, cast to bf16 by the SWDGE dma; two
                    # halves so the og's first k-groups can start while the
                    # second half is still streaming
                    wg = wpool.tile([P, NK * OGW], BF16, tag="wg")
                    half = (NK // 2) * OGW
                    rows = slice((og - 2) * P, (og - 1) * P)
                    nc.gpsimd.dma_start(wg[:, :half], wtu8_d[rows, :half])
                    nc.gpsimd.dma_start(wg[:, half:], wtu8_d[rows, half:])
                    wg_tiles[og] = wg
                osl = slice(og * OGW, (og + 1) * OGW)
                wg = wg_tiles[og]
                ps = mmps.tile([P, OGW], F32, tag="ps")
                for k in range(NK):
                    nc.tensor.matmul(ps[:], at_tiles[tt][:, k, :],
                                     wg[:, k * OGW:(k + 1) * OGW],
                                     start=(k == 0), stop=(k == NK - 1))
                # c1 = ps - Asum*z ; y = (c1*sc)*s  (bf16 epilogue tiles:
                # c1 rounding ~2e-3 rel, well inside the 2e-2 gate, and
                # 16-bit operands speed up the DVE)
                c1 = cpool.tile([P, OGW], BF16, tag="c1")
                nc.vector.scalar_tensor_tensor(
                    c1[:], zbs_tiles[og][:], nas_tiles[tt][:], ps[:],
                    op0=OP.mult, op1=OP.add)
                y2 = ypool.tile([P, OGW], BF16, tag="y2")
                nc.vector.scalar_tensor_tensor(
                    y2[:], c1[:], sc_tiles[tt][:], sbs_tiles[og][:],
                    op0=OP.mult, op1=OP.mult)
                nc.scalar.dma_start(y_t[tt, :, osl], y2[:])

            # ---------------- emission order ----------------
            # The PE queue must interleave the early tiles' PE transposes
            # with the first matmul groups (in-order queue: a transpose
            # emitted too early would stall matmuls behind its readiness).
            # Phase 1 runs og0/og1 PAIRED per t-tile, which pushes every
            # at-tile's deadline one 7us slot later than the lead-lag
            # order and lets the quant pipeline absorb transpose latency.
            quant_tile(0, do_transpose=True)
            quant_tile(1, do_transpose=False)
            matmul_group(0, 0)
            emit_transpose(1)
            quant_tile(2, do_transpose=False)
            matmul_group(1, 0)
            emit_transpose(2)
            quant_tile(3, do_transpose=False)
            matmul_group(0, 1)
            emit_transpose(3)
            quant_tile(4, do_transpose=False)
            matmul_group(1, 1)
            emit_transpose(4)
            for tt in range(5, NT):
                quant_tile(tt, do_transpose=True)

            # og0 finishes two slots early ((0,7) hoisted before (1,6)) so
            # og2's weight dma (ring-gated on og0's tile) hides behind
            # og1's tail groups.
            seq = []
            for tt in range(2, NT - 1):
                seq += [(0, tt), (1, tt)]
            seq.remove((1, NT - 2))
            seq += [(0, NT - 1), (1, NT - 2), (1, NT - 1)]
            for og in range(2, NOG):
                seq += [(og, tt) for tt in range(NT)]
            for og, tt in seq:
                matmul_group(og, tt)

    nc.compile()
    return nc


_NC = None


def _get_nc():
    global _NC
    if _NC is None:
        _NC = _build_nc()
    return _NC


def _prep_inputs(x, weight, scales, zeros):
    x2 = np.ascontiguousarray(x.reshape(NCORES * T, IN).astype(np.float32))
    w4 = weight.astype(np.float32).reshape(NOG, OGW, NK, P)
    wt = np.ascontiguousarray(
        w4.transpose(0, 3, 2, 1).reshape(NOG * P, NK * OGW))
    wt01 = wt[:2 * P].astype(ml_dtypes.bfloat16)
    wtu8 = (wt[2 * P:] + 128.0).astype(np.uint8)
    sb = np.ascontiguousarray(np.broadcast_to(
        scales.astype(np.float32), (P, OUT))).astype(ml_dtypes.bfloat16)
    zadj = zeros.astype(np.float32).copy()
    zadj[2 * OGW:] += 128.0
    zb = np.ascontiguousarray(
        np.broadcast_to(zadj, (P, OUT))).astype(ml_dtypes.bfloat16)
    ident = np.eye(P, dtype=ml_dtypes.bfloat16)
    in_maps = []
    for c in range(NCORES):
        in_maps.append({
            "x": np.ascontiguousarray(x2[c * T:(c + 1) * T]),
            "wt01": wt01,
            "wtu8": wtu8,
            "sb": sb,
            "zb": zb,
            "ident": ident,
        })
    return in_maps


def _run(x, weight, scales, zeros, trace=False):
    nc = _get_nc()
    in_maps = _prep_inputs(x, weight, scales, zeros)
    bkr = run_bass_kernel_spmd(nc, in_maps, core_ids=list(range(NCORES)),
                               trace=trace)
    y = np.concatenate([np.asarray(r["y"]) for r in bkr.results], axis=0)
    y = y.astype(np.float32).reshape(4, 2048, OUT)
    return y, bkr


def kernel(x, weight, scales, zeros):
    y, _ = _run(x, weight, scales, zeros, trace=False)
    return y
